# revision 1
# baseline (speedup 1.0000x reference)
"""Trainium2 Bass kernel for nn_Discriminator (DCRNN-style GRU discriminator).

Math restructure (exactly equivalent to the reference):
  dconv(xh, A, W, b) with xh=[x, h] splits into x-terms and h-terms:
    out_g = x W0x + (A x) W1x + (A^2 x) W2x        (precomputed per block, all T at once)
          + h W0h + (A h) W1h + (A^2 h) W2h + bg   (per step)
  A^2 is precomputed on host; A-side matmuls run in bf16 (error ~1e-5),
  feature-contraction (gate) matmuls run in fp32r (tf32-class), elementwise fp32.

Sharding: data-parallel over batch B=4 (cores 4-7 duplicate batches 0-3).
Each core runs both GRU blocks for its batch; host does the final tiny
pred = H[:,-1] @ W_sn + b_out and the mean, in float64.
"""
import numpy as np
import ml_dtypes

import concourse.bass as bass
import concourse.mybir as mybir
import concourse.tile as tile
from concourse import bacc
from concourse import bass_utils
from concourse.masks import make_identity

FP32 = mybir.dt.float32
FP32R = mybir.dt.float32r
BF16 = mybir.dt.bfloat16
AF = mybir.ActivationFunctionType

B, T, N, DIN, DH, K, NBLK = 4, 8, 2048, 64, 64, 3, 2
NC = N // 128            # 16 node chunks (full graph)
NO = 1024                # nodes owned per core
NCO = NO // 128          # 8 owned chunks
NJ = NO // 512           # 2 big column chunks over owned nodes
G = 2 * DH               # 128 gate width


def build_kernel(repeat=1, debug=False, dbg_blk=0, dbg_t=1, trace_sim=False):
    nc = bacc.Bacc(None, target_bir_lowering=False)

    # ---------------- I/O ----------------
    # A transposed / A^2 transposed, bf16 [N, N] (row m = source node)
    AT_d = nc.dram_tensor("AT", [N, NO], BF16, kind="ExternalInput")
    A2T_d = nc.dram_tensor("A2T", [N, NO], BF16, kind="ExternalInput")
    # X node-major stacked features, chunk layout [128, NC*T*DIN] bf16,
    # col c*512 + t*64 + f  <->  X[t, c*128+p, f]
    XF_d = nc.dram_tensor("XF", [128, T * NC * DIN], BF16, kind="ExternalInput")
    # X feat-major [T*DIN, N] fp32 (row t*64+f, col n)
    XT_d = nc.dram_tensor("XT", [T * DIN, NO], BF16, kind="ExternalInput")
    # weights (host spectral-normalized, split, padded):
    # g-path: WGH [NBLK, 65, 128] (rows 0:64 = W0h', row 64 = bg), fp32r
    WGH_d = nc.dram_tensor("WGH", [NBLK, DH + 1, G], BF16, kind="ExternalInput")
    WGH12_d = nc.dram_tensor("WGH12", [NBLK, 2 * DH, G], BF16, kind="ExternalInput")
    WGX0_d = nc.dram_tensor("WGX0", [NBLK, DIN, G], BF16, kind="ExternalInput")
    WGX12_d = nc.dram_tensor("WGX12", [NBLK, 2 * DIN, G], BF16, kind="ExternalInput")
    # c-path
    WCH_d = nc.dram_tensor("WCH", [NBLK, DH + 1, DH], BF16, kind="ExternalInput")
    WCH12_d = nc.dram_tensor("WCH12", [NBLK, 2 * DH, DH], BF16, kind="ExternalInput")
    WCX0_d = nc.dram_tensor("WCX0", [NBLK, DIN, DH], BF16, kind="ExternalInput")
    WCX12_d = nc.dram_tensor("WCX12", [NBLK, 2 * DIN, DH], BF16, kind="ExternalInput")

    HOUT_d = nc.dram_tensor("HOUT", [128, NCO * DH], FP32, kind="ExternalOutput")
    assert not debug, "v2 has no debug dumps"

    with tile.TileContext(nc, trace_sim=trace_sim) as tc:
        with (
            tc.tile_pool(name="big", bufs=1) as big,          # A matrices, persistent
            tc.tile_pool(name="wpool", bufs=1) as wpool,      # weights, identity
            tc.tile_pool(name="state", bufs=2) as state,      # h, h_bf
            tc.tile_pool(name="work", bufs=1) as work,        # hT/rhT/S12/g/rh/cc
            tc.tile_pool(name="stream", bufs=2) as stream,    # XT_t, P12_t, lhsT_mp
            tc.tile_pool(name="scr", bufs=1) as scr,          # elementwise temps
            tc.tile_pool(name="ptr", bufs=2, space="PSUM") as ptr,  # transpose psum
            tc.tile_pool(name="ps12", bufs=2, space="PSUM") as ps12,  # A-mult + precompute psum
            tc.tile_pool(name="pg", bufs=2, space="PSUM") as pg,    # gate psum
            tc.tile_pool(name="dram", bufs=1, space="DRAM") as dram,
        ):
            # ---------- persistent SBUF ----------
            AT_s = big.tile([128, NC * NO], BF16)    # [p, c*NO + x(own)]
            A2T_s = big.tile([128, NC * NO], BF16)
            for src_d, dst in ((AT_d, AT_s), (A2T_d, A2T_s)):
                # dst[p, c*NO + x] = src[c*128 + p, x]
                nc.sync.dma_start(
                    dst[:].rearrange("p (c x) -> p c x", c=NC),
                    src_d[:].rearrange("(c p) x -> p c x", c=NC),
                )

            ident = wpool.tile([128, 128], FP32)
            make_identity(nc, ident[:])
            ident_bf = wpool.tile([128, 128], BF16)
            nc.vector.tensor_copy(ident_bf[:], ident[:])

            # weights to SBUF (one tile per block; SBUF dim0 = partitions)
            def wtiles(dram_t, p, f, dt, nm):
                ts = []
                for blk in range(NBLK):
                    tl = wpool.tile([p, f], dt, name=f"{nm}{blk}", tag=f"{nm}{blk}")
                    nc.sync.dma_start(tl[:], dram_t[blk])
                    ts.append(tl)
                return ts
            wgh = wtiles(WGH_d, DH + 1, G, BF16, "wgh")
            wgh12 = wtiles(WGH12_d, 2 * DH, G, BF16, "wgh12")
            wgx0 = wtiles(WGX0_d, DIN, G, BF16, "wgx0")
            wgx12 = wtiles(WGX12_d, 2 * DIN, G, BF16, "wgx12")
            wch = wtiles(WCH_d, DH + 1, DH, BF16, "wch")
            wch12 = wtiles(WCH12_d, 2 * DH, DH, BF16, "wch12")
            wcx0 = wtiles(WCX0_d, DIN, DH, BF16, "wcx0")
            wcx12 = wtiles(WCX12_d, 2 * DIN, DH, BF16, "wcx12")

            # hT / rhT tiles with a persistent ones-row (row 64) for the bias
            hT = work.tile([DH + 1, NO], BF16)
            ones32 = wpool.tile([1, 512], FP32)
            nc.gpsimd.memset(ones32[:], 1.0)
            for q in range(2):
                nc.vector.tensor_copy(hT[DH:DH + 1, q * 512:(q + 1) * 512], ones32[:])
            rhT = hT

            # staging DRAM for P1^T/P2^T (bf16) and block-2 inputs
            P1T_dr = dram.tile([T * DIN, NO], BF16)
            P2T_dr = dram.tile([T * DIN, NO], BF16)
            H1T_dr = dram.tile([T * DH, NO], BF16)
            H1G_dr = dram.tile([T * N, DH], BF16)
            AGIN_h = dram.tile([NO, DH], BF16)
            AGIN_rh = dram.tile([NO, DH], BF16)
            RHG_a = dram.tile([N, DH], BF16)
            RHG_b = dram.tile([N, DH], BF16)
            HG2_a = dram.tile([N, DH], BF16)
            HG2_b = dram.tile([N, DH], BF16)
            RG = [[b, b + 4] for b in range(4)]

            def dump_p(blk):
                pass

            def load_lhsT_xf(lhsT, mp):
                # lhsT[p, c*128 + tt*64 + f] = XF[p, (2*mp+tt)*1024 + c*64 + f]
                for tt in range(2):
                    nc.sync.dma_start(
                        lhsT[:].rearrange("p (c j) -> p c j", c=NC)[:, :, tt * 64:(tt + 1) * 64],
                        XF_d[:, (2 * mp + tt) * (NC * DIN):(2 * mp + tt + 1) * (NC * DIN)]
                            .rearrange("p (c f) -> p c f", c=NC),
                    )

            def load_lhsT_h1g(lhsT, mp):
                # lhsT[p, c*128 + tt*64 + f] = H1G[(2*mp+tt)*N + c*128 + p, f]
                for tt in range(2):
                    t_ = 2 * mp + tt
                    nc.sync.dma_start(
                        lhsT[:].rearrange("p (c j) -> p c j", c=NC)[:, :, tt * 64:(tt + 1) * 64],
                        H1G_dr[t_ * N:(t_ + 1) * N, :].rearrange("(c p) f -> p c f", c=NC),
                    )

            def precompute(blk, loader):
                """P1^T = (A @ Xf)^T, P2^T = (A^2 @ Xf)^T (own cols) -> DRAM bf16."""
                for mp in range(4):  # M-pass: rows mp*128..mp*128+127 of P^T
                    lhsT = stream.tile([128, NC * 128], BF16, tag="p12", name="lhsT")
                    loader(lhsT, mp)
                    for src, pdst in ((AT_s, P1T_dr), (A2T_s, P2T_dr)):
                        for j in range(NJ):
                            ps = ps12.tile([128, 512], FP32, tag="s12p", name="pp")
                            for c in range(NC):
                                nc.tensor.matmul(
                                    ps[:],
                                    lhsT[:, c * 128:(c + 1) * 128],
                                    src[:, c * NO + j * 512: c * NO + j * 512 + 512],
                                    start=(c == 0), stop=(c == NC - 1),
                                )
                            st = scr.tile([128, 512], BF16, tag="rhbf", name="pstg")
                            nc.vector.tensor_copy(st[:], ps[:])
                            nc.sync.dma_start(
                                pdst[mp * 128:(mp + 1) * 128, j * 512:(j + 1) * 512], st[:])

            def transpose_to(dst_T, src_bf, nrows):
                """PE-transpose bf16 src [128, NCO*nrows] (chunk-major) -> dst_T [nrows, NO]."""
                for q in range(2):
                    pt = ptr.tile([nrows, 512], BF16, tag="ptr", name="ptr")
                    for ci in range(4):
                        c = q * 4 + ci
                        nc.tensor.transpose(
                            pt[:, ci * 128:(ci + 1) * 128],
                            src_bf[:, c * nrows:(c + 1) * nrows],
                            ident_bf[:],
                        )
                    nc.vector.tensor_copy(dst_T[0:nrows, q * 512:(q + 1) * 512], pt[:])

            def gru_block(blk, xt_dram, p1_dram, p2_dram, store_h1):
                # initial state h = 0 (own nodes)
                h = state.tile([128, NCO * DH], FP32, tag="h", name="h")
                nc.gpsimd.memset(h[:], 0.0)

                def gather(src_bf_own, gout_ap, agin):
                    """AllGather own node-major rows -> full [N, DH] bf16 dram."""
                    nc.sync.dma_start(
                        agin[:].rearrange("(ch p) f -> p ch f", ch=NCO),
                        src_bf_own[:].rearrange("p (ch f) -> p ch f", ch=NCO),
                    )
                    nc.gpsimd.collective_compute(
                        "AllGather", mybir.AluOpType.bypass,
                        ins=[agin[:]], outs=[gout_ap],
                        replica_groups=RG,
                    )

                def to_sbuf_full(gout_ap):
                    hf = stream.tile([128, NC * DH], BF16, tag="hfull", name="hfull")
                    nc.sync.dma_start(
                        hf[:].rearrange("p (c f) -> p c f", c=NC),
                        gout_ap.rearrange("(c p) f -> p c f", c=NC),
                    )
                    return hf

                def amult(hf, tag):
                    """S12^T (own cols) from full node-major bf16 lhsT."""
                    s12_ = work.tile([128, NO], BF16, tag="s12", name=tag)
                    for j in range(NJ):
                        ps = ps12.tile([128, 512], FP32, tag="s12p", name="s12p" + tag)
                        for c in range(NC):
                            lh = hf[:, c * DH:(c + 1) * DH]
                            nc.tensor.matmul(
                                ps[0:DH, :], lh,
                                AT_s[:, c * NO + j * 512: c * NO + j * 512 + 512],
                                start=(c == 0), stop=(c == NC - 1))
                            nc.tensor.matmul(
                                ps[DH:128, :], lh,
                                A2T_s[:, c * NO + j * 512: c * NO + j * 512 + 512],
                                start=(c == 0), stop=(c == NC - 1))
                        nc.vector.tensor_copy(s12_[:, j * 512:(j + 1) * 512], ps[:])
                    return s12_

                for t in range(T):
                    # --- stream x-term slices (own nodes) ---
                    xt_t = stream.tile([DIN, NO], BF16, tag="xt", name="xt")
                    nc.sync.dma_start(xt_t[:], xt_dram[t * DIN:(t + 1) * DIN, :])
                    p12_t = stream.tile([128, NO], BF16, tag="p12", name="p12")
                    nc.sync.dma_start(p12_t[0:DIN, :], p1_dram[t * DIN:(t + 1) * DIN, :])
                    nc.sync.dma_start(p12_t[DIN:128, :], p2_dram[t * DIN:(t + 1) * DIN, :])

                    # --- g path ---
                    if t > 0:
                        # gather current h (result of step t-1) across the pair;
                        # issue the collective before local transposes
                        hbf = scr.tile([128, NCO * DH], BF16, tag="hbfg", name="hbf")
                        nc.vector.tensor_copy(hbf[:], h[:])
                        if store_h1:
                            gout = H1G_dr[(t - 1) * N:t * N, :]
                        else:
                            gout = (HG2_a if t % 2 else HG2_b)[:]
                        gather(hbf, gout, AGIN_h)
                        transpose_to(hT, hbf, DH)
                        if store_h1:
                            nc.sync.dma_start(H1T_dr[(t - 1) * DH:t * DH, :], hT[0:DH, :])
                        hf = to_sbuf_full(gout)
                        s12 = amult(hf, "s12g")

                    g = work.tile([128, NCO * G], FP32, tag="g", name="g")
                    for cg in range(2):  # groups of 4 node-chunks -> one psum bank
                        psg = pg.tile([128, 512], FP32, tag="pg", name="pg")
                        for ci in range(4):
                            c = cg * 4 + ci
                            o = psg[:, ci * 128:(ci + 1) * 128]
                            sl = slice(c * 128, (c + 1) * 128)
                            if t > 0:
                                nc.tensor.matmul(o, hT[:, sl], wgh[blk][:], start=True, stop=False)
                                nc.tensor.matmul(o, s12[:, sl], wgh12[blk][:], start=False, stop=False)
                            else:
                                nc.tensor.matmul(o, hT[DH:DH + 1, sl], wgh[blk][DH:DH + 1, :], start=True, stop=False)
                            nc.tensor.matmul(o, xt_t[:, sl], wgx0[blk][:], start=False, stop=False)
                            nc.tensor.matmul(o, p12_t[:, sl], wgx12[blk][:], start=False, stop=True)
                        nc.scalar.activation(g[:, cg * 512:(cg + 1) * 512], psg[:], AF.Sigmoid)

                    # rh = r * h (own)
                    if t > 0:
                        rh = scr.tile([128, NCO * DH], FP32, tag="rh", name="rh")
                        r_view = g[:].rearrange("p (c f) -> p c f", c=NCO)[:, :, 0:DH]
                        h_view = h[:].rearrange("p (c f) -> p c f", c=NCO)
                        rh_view = rh[:].rearrange("p (c f) -> p c f", c=NCO)
                        nc.vector.tensor_mul(rh_view, r_view, h_view)

                    # --- c path ---
                    if t > 0:
                        rhbf = scr.tile([128, NCO * DH], BF16, tag="rhbf", name="rhbf")
                        nc.vector.tensor_copy(rhbf[:], rh[:])
                        rhg = (RHG_a if t % 2 else RHG_b)[:]
                        gather(rhbf, rhg, AGIN_rh)
                        transpose_to(rhT, rhbf, DH)
                        rhf = to_sbuf_full(rhg)
                        s12c = amult(rhf, "s12c")

                    cc = scr.tile([128, NCO * DH], FP32, tag="cc", name="cc")
                    psc = pg.tile([128, 512], FP32, tag="pg", name="pgc")
                    for ci in range(8):
                        o = psc[:, ci * DH:(ci + 1) * DH]
                        sl = slice(ci * 128, (ci + 1) * 128)
                        if t > 0:
                            nc.tensor.matmul(o, rhT[:, sl], wch[blk][:], start=True, stop=False)
                            nc.tensor.matmul(o, s12c[:, sl], wch12[blk][:], start=False, stop=False)
                        else:
                            nc.tensor.matmul(o, rhT[DH:DH + 1, sl], wch[blk][DH:DH + 1, :], start=True, stop=False)
                        nc.tensor.matmul(o, xt_t[:, sl], wcx0[blk][:], start=False, stop=False)
                        nc.tensor.matmul(o, p12_t[:, sl], wcx12[blk][:], start=False, stop=True)
                    nc.scalar.activation(cc[:], psc[:], AF.Tanh)

                    # h_new = cc + u * (h - cc)
                    u_view = g[:].rearrange("p (c f) -> p c f", c=NCO)[:, :, DH:G]
                    hmc = scr.tile([128, NCO * DH], FP32, tag="rh", name="hmc")
                    nc.vector.tensor_sub(hmc[:], h[:], cc[:])
                    h_new = state.tile([128, NCO * DH], FP32, tag="h", name="hn")
                    hmc_view = hmc[:].rearrange("p (c f) -> p c f", c=NCO)
                    nc.vector.tensor_mul(hmc_view, u_view, hmc_view)
                    nc.vector.tensor_add(h_new[:], cc[:], hmc[:])
                    h = h_new

                if store_h1:
                    # final h: gather for H1G + transpose for H1T
                    hbf = scr.tile([128, NCO * DH], BF16, tag="hbfg", name="hbff")
                    nc.vector.tensor_copy(hbf[:], h[:])
                    gather(hbf, H1G_dr[(T - 1) * N:T * N, :], AGIN_h)
                    transpose_to(hT, hbf, DH)
                    nc.sync.dma_start(H1T_dr[(T - 1) * DH:T * DH, :], hT[0:DH, :])
                return h

            for _rep in range(repeat):
                precompute(0, load_lhsT_xf)
                dump_p(0)
                gru_block(0, XT_d, P1T_dr, P2T_dr, store_h1=True)
                precompute(1, load_lhsT_h1g)
                dump_p(1)
                h_fin = gru_block(1, H1T_dr, P1T_dr, P2T_dr, store_h1=False)

            nc.sync.dma_start(HOUT_d[:], h_fin[:])

    nc.finalize()
    return nc


# ---------------------------------------------------------------------------
# host-side preparation and execution
# ---------------------------------------------------------------------------

def _prep_inputs(X, A_x, Wg, bg, Wc, bc):
    f32 = np.float32
    A = A_x.astype(np.float64)
    A2 = A @ A
    AT = np.ascontiguousarray(A.T.astype(ml_dtypes.bfloat16))
    A2T = np.ascontiguousarray(A2.T.astype(ml_dtypes.bfloat16))

    def spec_norm(W):
        M = W.reshape(-1, W.shape[-1]).astype(np.float64)
        sigma = np.linalg.norm(M, ord=2)
        return (W.astype(np.float64) / sigma).astype(f32)

    WGH = np.zeros((NBLK, DH + 1, G), f32)
    WGH12 = np.zeros((NBLK, 2 * DH, G), f32)
    WGX0 = np.zeros((NBLK, DIN, G), f32)
    WGX12 = np.zeros((NBLK, 2 * DIN, G), f32)
    WCH = np.zeros((NBLK, DH + 1, DH), f32)
    WCH12 = np.zeros((NBLK, 2 * DH, DH), f32)
    WCX0 = np.zeros((NBLK, DIN, DH), f32)
    WCX12 = np.zeros((NBLK, 2 * DIN, DH), f32)
    for blk in range(NBLK):
        Wg_n = spec_norm(Wg[blk])
        Wc_n = spec_norm(Wc[blk])
        WGX0[blk] = Wg_n[0][:DIN]
        WGH[blk, :DH] = Wg_n[0][DIN:]
        WGH[blk, DH] = bg[blk]
        WGX12[blk, :DIN] = Wg_n[1][:DIN]
        WGX12[blk, DIN:] = Wg_n[2][:DIN]
        WGH12[blk, :DH] = Wg_n[1][DIN:]
        WGH12[blk, DH:] = Wg_n[2][DIN:]
        WCX0[blk] = Wc_n[0][:DIN]
        WCH[blk, :DH] = Wc_n[0][DIN:]
        WCH[blk, DH] = bc[blk]
        WCX12[blk, :DIN] = Wc_n[1][:DIN]
        WCX12[blk, DIN:] = Wc_n[2][:DIN]
        WCH12[blk, :DH] = Wc_n[1][DIN:]
        WCH12[blk, DH:] = Wc_n[2][DIN:]

    shared = {
        "WGH": WGH.astype(ml_dtypes.bfloat16), "WGH12": WGH12.astype(ml_dtypes.bfloat16),
        "WGX0": WGX0.astype(ml_dtypes.bfloat16), "WGX12": WGX12.astype(ml_dtypes.bfloat16),
        "WCH": WCH.astype(ml_dtypes.bfloat16), "WCH12": WCH12.astype(ml_dtypes.bfloat16),
        "WCX0": WCX0.astype(ml_dtypes.bfloat16), "WCX12": WCX12.astype(ml_dtypes.bfloat16),
    }

    in_maps = []
    for core in range(8):
        b = core % B
        half = core // B
        own = slice(half * NO, (half + 1) * NO)
        Xb = X[b]                                    # [T, N, DIN]
        XF = np.ascontiguousarray(
            Xb.reshape(T, NC, 128, DIN).transpose(2, 0, 1, 3).reshape(128, T * NC * DIN)
        ).astype(ml_dtypes.bfloat16)
        XT = np.ascontiguousarray(
            Xb.transpose(0, 2, 1).reshape(T * DIN, N)[:, own]).astype(ml_dtypes.bfloat16)
        im = dict(shared)
        im["AT"] = np.ascontiguousarray(AT[:, own])
        im["A2T"] = np.ascontiguousarray(A2T[:, own])
        im["XF"] = XF
        im["XT"] = XT
        in_maps.append(im)
    return in_maps


_CACHED = {}


def _get_nc(repeat=1, debug=False, dbg_blk=0, dbg_t=1):
    key = (repeat, debug, dbg_blk, dbg_t)
    if key not in _CACHED:
        _CACHED[key] = build_kernel(repeat, debug, dbg_blk, dbg_t)
    return _CACHED[key]


def run_on_device(inputs, repeat=1, time_iters=0, debug=False, dbg_blk=0, dbg_t=1, raw=False,
                  use_spmd_api=False):
    """Returns (per-batch final h [B, N, DH] fp32, wall_ns or None)."""
    nc = _get_nc(repeat, debug, dbg_blk, dbg_t)
    in_maps = _prep_inputs(inputs["X"], inputs["A_x"], inputs["Wg"], inputs["bg"],
                           inputs["Wc"], inputs["bc"])
    if use_spmd_api:
        from concourse import bass_utils as _bu
        res = _bu.run_bass_kernel_spmd(nc, in_maps, core_ids=list(range(8)), trace=False)
        results, wall = res.results, None
    else:
        from runner_embedded import make_runner
        run = make_runner(nc, 8)
        results, wall = run(in_maps, time_iters=time_iters)
    if raw:
        return results, wall
    hs = []
    for b in range(B):
        lo = results[b]["HOUT"].reshape(128, NCO, DH).transpose(1, 0, 2).reshape(NO, DH)
        hi = results[b + 4]["HOUT"].reshape(128, NCO, DH).transpose(1, 0, 2).reshape(NO, DH)
        hs.append(np.concatenate([lo, hi], axis=0))
    return np.stack(hs), wall


def kernel(**inputs):
    X = inputs["X"]
    W_out = inputs["W_out"].astype(np.float64)
    b_out = inputs["b_out"].astype(np.float64)
    hs, _ = run_on_device(inputs, use_spmd_api=True)
    W_sn = W_out / np.linalg.norm(W_out)
    pred = hs.astype(np.float64) @ W_sn + b_out     # [B, N, 1]
    out = pred.squeeze(-1).mean()
    return np.float32(out)


# ---- embedded runner (kernel.py must be self-contained) ----
import sys as _sys
import types as _types

_runner_src = '''
import time
import numpy as np
import jax
from jax.sharding import Mesh, PartitionSpec
from jax.experimental.shard_map import shard_map

import concourse.mybir as mybir
from concourse.bass2jax import _bass_exec_p, partition_id_tensor, install_neuronx_cc_hook


def make_runner(nc, n_cores):
    install_neuronx_cc_hook()
    partition_name = nc.partition_id_tensor.name if nc.partition_id_tensor else None

    in_names = []
    out_names = []
    out_avals = []
    zero_outs = []
    for alloc in nc.m.functions[0].allocations:
        if not isinstance(alloc, mybir.MemoryLocationSet):
            continue
        name = alloc.memorylocations[0].name
        if alloc.kind == "ExternalInput":
            if name != partition_name:
                in_names.append(name)
        elif alloc.kind == "ExternalOutput":
            out_names.append(name)
            shape = tuple(alloc.tensor_shape)
            dtype = mybir.dt.np(alloc.dtype)
            out_avals.append(jax.core.ShapedArray(shape, dtype))
            zero_outs.append(np.zeros(shape, dtype))
    n_params = len(in_names)
    n_outs = len(out_avals)
    all_in_names = list(in_names) + list(out_names)
    if partition_name is not None:
        all_in_names.append(partition_name)

    def _body(*args):
        operands = list(args)
        if partition_name is not None:
            operands.append(partition_id_tensor())
        outs = _bass_exec_p.bind(
            *operands,
            out_avals=tuple(out_avals),
            in_names=tuple(all_in_names),
            out_names=tuple(out_names),
            lowering_input_output_aliases=(),
            sim_require_finite=False,
            sim_require_nnan=False,
            nc=nc,
        )
        return tuple(outs)

    devices = jax.devices()[:n_cores]
    mesh = Mesh(np.asarray(devices), ("core",))
    in_specs = (PartitionSpec("core"),) * (n_params + n_outs)
    out_specs = (PartitionSpec("core"),) * len(out_names)
    sharded = jax.jit(
        shard_map(_body, mesh=mesh, in_specs=in_specs, out_specs=out_specs,
                  check_rep=False),
        keep_unused=True,
    )

    def run(in_maps, time_iters=0):
        per_core = [[np.asarray(m[name]) for name in in_names] for m in in_maps]
        concat_in = [
            np.concatenate([per_core[c][i] for c in range(n_cores)], axis=0)
            for i in range(n_params)
        ]
        concat_zeros = [
            np.zeros((n_cores * z.shape[0], *z.shape[1:]), z.dtype) for z in zero_outs
        ]
        out_arrs = sharded(*concat_in, *concat_zeros)
        jax.block_until_ready(out_arrs)
        wall_ns = None
        if time_iters:
            times = []
            for _ in range(time_iters):
                t0 = time.perf_counter_ns()
                out_arrs = sharded(*concat_in, *concat_zeros)
                jax.block_until_ready(out_arrs)
                times.append(time.perf_counter_ns() - t0)
            wall_ns = min(times)
        results = [
            {name: np.asarray(out_arrs[i]).reshape(n_cores, *out_avals[i].shape)[c]
             for i, name in enumerate(out_names)}
            for c in range(n_cores)
        ]
        return results, wall_ns

    return run
'''

_mod = _types.ModuleType("runner_embedded")
exec(_runner_src, _mod.__dict__)
_sys.modules["runner_embedded"] = _mod


if __name__ == "__main__":
    pass

